# revision 18
# baseline (speedup 1.0000x reference)
"""Multi-head self-attention on 8 Trainium2 NeuronCores (Bass/Tile).

Problem: x[4, 2048, 1024], 16 heads x 64 dim, fused QKV/attention/out-proj.

Sharding (no collectives): core c handles batch b = c//2 and query-half
qh = c%2 (1024 queries), all 16 heads. K/V are computed for the full 2048
tokens of batch b (2x redundancy within a batch pair); outputs are disjoint
[1024, 1024] slices that the host concatenates.

On-chip layout (all fp16 operands, fp32 PSUM accumulation):
  - x^T [1024 in, 2048 tok] per batch, query-half tokens permuted first
  - Q^T/K^T proj: out[feat 128, tok] tiles (feature-major => heads land on
    partitions, d=64), softmax scale 1/8 folded into wq/qb on host
  - scores^T [k 128, q 512] via row-packed head-pair matmuls (d=64 each)
  - exp on ScalarE straight out of PSUM (no max subtraction: |s|/8 <~ 3)
  - P^T accumulated across k-chunks on DVE (fp16), row-sums via ones-matmul,
    reciprocal broadcast back to 128 partitions via a rank-1 matmul
  - PV col-packed per head pair -> A^T [128 feat, q], divided during PSUM
    evacuation
  - out-proj contracts the 8 A^T pair-chunks, bias added on DVE, fp32 out
"""

import numpy as np

EMBED = 1024
NH = 16
D = 64
B = 4
T = 2048
TQ = 1024  # queries per core
NCORES = 8
NIC = EMBED // 128  # 8 contraction chunks
NHP = NH // 2  # 8 head pairs

_PROGRAM = None


def _build_program():
    import concourse.bass as bass
    import concourse.mybir as mybir
    import concourse.tile as tile
    from concourse import bacc

    F16 = mybir.dt.float16
    F32 = mybir.dt.float32
    AF = mybir.ActivationFunctionType

    nc = bacc.Bacc("TRN2", target_bir_lowering=False, debug=False,
                   num_devices=NCORES)

    xT_d = nc.dram_tensor("xT", [EMBED, T], F16, kind="ExternalInput").ap()
    wq_d = nc.dram_tensor("wqT", [EMBED, EMBED], F16, kind="ExternalInput").ap()
    wk_d = nc.dram_tensor("wkT", [EMBED, EMBED], F16, kind="ExternalInput").ap()
    wv_d = nc.dram_tensor("wvT", [EMBED, EMBED], F16, kind="ExternalInput").ap()
    wo_d = nc.dram_tensor("woT", [EMBED, EMBED], F16, kind="ExternalInput").ap()
    qb_d = nc.dram_tensor("qb", [128, NIC], F32, kind="ExternalInput").ap()
    kb_d = nc.dram_tensor("kb", [128, NIC], F32, kind="ExternalInput").ap()
    vbb_d = nc.dram_tensor("vbb", [128, EMBED], F16, kind="ExternalInput").ap()
    obb_d = nc.dram_tensor("obb", [128, EMBED], F32, kind="ExternalInput").ap()
    ones_d = nc.dram_tensor("ones", [128, 1], F16, kind="ExternalInput").ap()
    sel_d = nc.dram_tensor("sel", [1, 256], F16, kind="ExternalInput").ap()
    y_d = nc.dram_tensor("y", [TQ, EMBED], F32, kind="ExternalOutput").ap()

    # pair-exchange staging: each core computes K/V for its own 1024 tokens
    # and AllGathers with its batch partner (replica groups of 2)
    PAIRS = [[2 * g, 2 * g + 1] for g in range(NCORES // 2)]
    kxb_d = [nc.dram_tensor(f"kxb{h}", [128, TQ], F16).ap() for h in range(NHP)]
    kg_d = [nc.dram_tensor(f"kg{h}", [2, 128, TQ], F16).ap()
            for h in range(NHP)]
    vxb_d = [nc.dram_tensor(f"vxb{o}", [TQ, 512], F16).ap() for o in range(2)]
    vg_d = [nc.dram_tensor(f"vg{o}", [2, TQ, 512], F16).ap() for o in range(2)]

    xT_r = xT_d.rearrange("(c p) t -> c p t", p=128)
    wq_r = wq_d.rearrange("(c p) o -> c p o", p=128)
    wk_r = wk_d.rearrange("(c p) o -> c p o", p=128)
    wv_r = wv_d.rearrange("(c p) o -> c p o", p=128)
    wo_r = wo_d.rearrange("(c p) o -> c p o", p=128)
    y_r = y_d.rearrange("(tb p) o -> tb p o", p=128)

    NKC = T // 128       # 16 key chunks
    NQB = TQ // 512      # 2 query blocks
    NTB = T // 128       # 16 token blocks for V
    NVO = 2              # V out-feature 512-blocks

    with tile.TileContext(nc) as tc:
        from contextlib import ExitStack
        with ExitStack() as ctx:
            cst = ctx.enter_context(tc.tile_pool(name="cst", bufs=1))
            big = ctx.enter_context(tc.tile_pool(name="big", bufs=1))
            wqk = ctx.enter_context(tc.tile_pool(name="wqk", bufs=2))
            qkp = ctx.enter_context(tc.tile_pool(name="qkp", bufs=2))
            pTp = ctx.enter_context(tc.tile_pool(name="pTp", bufs=8))
            accp = ctx.enter_context(tc.tile_pool(name="accp", bufs=2))
            misc = ctx.enter_context(tc.tile_pool(name="misc", bufs=2))
            outp = ctx.enter_context(tc.tile_pool(name="outp", bufs=3))
            ps_st = ctx.enter_context(
                tc.tile_pool(name="ps_st", bufs=2, space="PSUM"))
            ps_pv = ctx.enter_context(
                tc.tile_pool(name="ps_pv", bufs=2, space="PSUM"))
            ps_sm = ctx.enter_context(
                tc.tile_pool(name="ps_sm", bufs=2, space="PSUM"))

            # ---- persistent tiles ----
            xT = big.tile([128, NIC * T], F16, tag="xT")          # 32KB/par
            wv = big.tile([128, NIC * EMBED], F16, tag="wv")      # 16KB
            wo = big.tile([128, NIC * EMBED], F16, tag="wo")      # 16KB
            vv = big.tile([128, NTB * EMBED], F16, tag="vv")      # 32KB
            aT = big.tile([128, NHP * TQ], F16, tag="aT")         # 16KB
            qb_sb = cst.tile([128, NIC], F32, tag="qb")
            kb_sb = cst.tile([128, NIC], F32, tag="kb")
            vbb = cst.tile([128, EMBED], F16, tag="vbb")
            obb = cst.tile([128, EMBED], F32, tag="obb")
            ones = cst.tile([128, 1], F16, tag="ones")
            sel = cst.tile([1, 256], F16, tag="sel")

            nc.sync.dma_start(qb_sb[:], qb_d[:])
            nc.sync.dma_start(kb_sb[:], kb_d[:])
            nc.sync.dma_start(vbb[:], vbb_d[:])
            nc.sync.dma_start(obb[:], obb_d[:])
            nc.sync.dma_start(ones[:], ones_d[:])
            nc.sync.dma_start(sel[:], sel_d[:])
            for c in range(NIC):
                nc.sync.dma_start(xT[:, c * T:(c + 1) * T], xT_r[c])
                nc.sync.dma_start(wv[:, c * EMBED:(c + 1) * EMBED], wv_r[c])

            def v_proj_tb(ob, tb):
                # V[tok, feat] for OWN token block tb (<8), feat block ob*512
                ps = ps_sm.tile([128, 512], F32, tag="small")
                for c in range(NIC):
                    nc.tensor.matmul(
                        ps[:],
                        lhsT=xT[:, c * T + tb * 128: c * T + tb * 128 + 128],
                        rhs=wv[:, c * EMBED + ob * 512: c * EMBED + ob * 512 + 512],
                        start=(c == 0), stop=(c == NIC - 1))
                vtmp = outp.tile([128, 512], F16, tag="vtmp")
                nc.vector.tensor_add(vtmp[:], ps[:],
                                     vbb[:, ob * 512:(ob + 1) * 512])
                nc.sync.dma_start(
                    vxb_d[ob][tb * 128:(tb + 1) * 128, :], vtmp[:])

            def v_gather(ob):
                nc.gpsimd.collective_compute(
                    "AllGather", mybir.AluOpType.bypass,
                    ins=[vxb_d[ob][:]], outs=[vg_d[ob][:]],
                    replica_groups=PAIRS)
                for g in range(2):
                    for tb in range(TQ // 128):
                        gt = g * (TQ // 128) + tb
                        nc.sync.dma_start(
                            vv[:, gt * EMBED + ob * 512:
                               gt * EMBED + ob * 512 + 512],
                            vg_d[ob][g, tb * 128:(tb + 1) * 128, :])

            # ---- per-head-pair K/Q projection, emitted one pair AHEAD,
            # interleaved into the previous pair's attention loop so the
            # scalar engine never drains between pairs ----
            kq = {}

            def alloc_kq(hp):
                wq_sb = wqk.tile([128, NIC * 128], F16, tag="wq")
                wk_sb = wqk.tile([128, NIC * 128], F16, tag="wk")
                for c in range(NIC):
                    nc.sync.dma_start(
                        wq_sb[:, c * 128:(c + 1) * 128],
                        wq_r[c][:, hp * 128:(hp + 1) * 128])
                    nc.sync.dma_start(
                        wk_sb[:, c * 128:(c + 1) * 128],
                        wk_r[c][:, hp * 128:(hp + 1) * 128])
                kT = qkp.tile([128, T], F16, tag="kT")
                qT = qkp.tile([128, TQ], F16, tag="qT")
                kTh = qkp.tile([128, TQ], F16, tag="kTh")
                kq[hp] = (wq_sb, wk_sb, kT, qT, kTh)

            def k_proj_tb(hp, tb):
                # K^T for OWN tokens only (tb < 2), then pair-exchange
                wq_sb, wk_sb, kT, qT, kTh = kq[hp]
                ps = ps_sm.tile([128, 512], F32, tag="small")
                for c in range(NIC):
                    nc.tensor.matmul(
                        ps[:], lhsT=wk_sb[:, c * 128:(c + 1) * 128],
                        rhs=xT[:, c * T + tb * 512: c * T + tb * 512 + 512],
                        start=(c == 0), stop=(c == NIC - 1))
                nc.vector.tensor_scalar_add(
                    kTh[:, tb * 512:(tb + 1) * 512], ps[:], kb_sb[:, hp:hp + 1])

            def k_gather(hp):
                _, _, kT, _, kTh = kq[hp]
                nc.sync.dma_start(kxb_d[hp][:], kTh[:])
                nc.gpsimd.collective_compute(
                    "AllGather", mybir.AluOpType.bypass,
                    ins=[kxb_d[hp][:]], outs=[kg_d[hp][:]],
                    replica_groups=PAIRS)
                nc.sync.dma_start(kT[:, 0:TQ], kg_d[hp][0])
                nc.sync.dma_start(kT[:, TQ:T], kg_d[hp][1])

            def q_proj_tb(hp, tb):
                wq_sb, wk_sb, kT, qT, kTh = kq[hp]
                ps = ps_sm.tile([128, 512], F32, tag="small")
                for c in range(NIC):
                    nc.tensor.matmul(
                        ps[:], lhsT=wq_sb[:, c * 128:(c + 1) * 128],
                        rhs=xT[:, c * T + tb * 512: c * T + tb * 512 + 512],
                        start=(c == 0), stop=(c == NIC - 1))
                nc.vector.tensor_scalar_add(
                    qT[:, tb * 512:(tb + 1) * 512], ps[:], qb_sb[:, hp:hp + 1])

            def o_proj_unit(tb, ob):
                ps = ps_sm.tile([128, 512], F32, tag="small")
                for f in range(NHP):
                    nc.tensor.matmul(
                        ps[:],
                        lhsT=aT[:, f * TQ + tb * 128: f * TQ + tb * 128 + 128],
                        rhs=wo[:, f * EMBED + ob * 512:
                               f * EMBED + ob * 512 + 512],
                        start=(f == 0), stop=(f == NHP - 1))
                out_sb = outp.tile([128, 512], F32, tag="out")
                nc.vector.tensor_add(out_sb[:], ps[:],
                                     obb[:, ob * 512:(ob + 1) * 512])
                nc.sync.dma_start(y_r[tb][:, ob * 512:(ob + 1) * 512],
                                  out_sb[:])

            alloc_kq(0)
            k_proj_tb(0, 0)
            k_proj_tb(0, 1)
            k_gather(0)
            q_proj_tb(0, 0)
            q_proj_tb(0, 1)
            for tb in range(TQ // 128):
                v_proj_tb(0, tb)
            v_gather(0)
            for c in range(NIC):
                nc.sync.dma_start(wo[:, c * EMBED:(c + 1) * EMBED], wo_r[c])

            for hp in range(NHP):
                _, _, kT, qT, _ = kq[hp]

                for qb in range(NQB):
                    if hp + 1 < NHP and qb == 0:
                        alloc_kq(hp + 1)
                    # interleave units: next-pair K/Q proj, the second V
                    # feature-block under hp1, and the first half of the
                    # out-projection under hp7 qb1
                    units = {}
                    if hp + 1 < NHP:
                        if qb == 0:
                            units = {4: (k_proj_tb, hp + 1, 0),
                                     12: (k_proj_tb, hp + 1, 1)}
                        else:
                            units = {2: (q_proj_tb, hp + 1, 0),
                                     10: (q_proj_tb, hp + 1, 1)}
                    elif qb == 1:
                        units = {2 * u + 1: (o_proj_unit, u // 2, u % 2)
                                 for u in range(8)}
                    if hp == 1 and qb == 0:
                        for u in range(TQ // 128):
                            units[2 * u + 1] = (v_proj_tb, 1, u)
                    pv = ps_pv.tile([128, 512], F32, tag="pv")
                    acc = accp.tile([128, 1024], F16, tag="acc")
                    prev_pT = None
                    for kc in range(NKC):
                        if kc in units:
                            fn, a0, a1 = units[kc]
                            fn(a0, a1)
                        st = ps_st.tile([128, 1024], F32, tag="st")
                        nc.tensor.matmul(
                            st[:, 0:512],
                            lhsT=kT[0:64, kc * 128:(kc + 1) * 128],
                            rhs=qT[0:64, qb * 512:(qb + 1) * 512],
                            start=True, stop=True)
                        nc.tensor.matmul(
                            st[:, 512:1024],
                            lhsT=kT[64:128, kc * 128:(kc + 1) * 128],
                            rhs=qT[64:128, qb * 512:(qb + 1) * 512],
                            start=True, stop=True, tile_position=(64, 0))
                        pT = pTp.tile([128, 1024], F16, tag="pT")
                        nc.scalar.activation(pT[:], st[:], AF.Exp)
                        with nc.allow_low_precision(
                                reason="fp16 softmax partial-sum accumulate"):
                            if kc == 1:
                                nc.vector.tensor_add(
                                    acc[:], prev_pT[:], pT[:])
                            elif kc > 1:
                                nc.vector.tensor_add(acc[:], acc[:], pT[:])
                        prev_pT = pT
                        nc.tensor.matmul(
                            pv[0:64, :],
                            lhsT=vv[:, kc * EMBED + hp * 128:
                                    kc * EMBED + hp * 128 + 64],
                            rhs=pT[:, 0:512],
                            start=(kc == 0), stop=(kc == NKC - 1))
                        nc.tensor.matmul(
                            pv[64:128, :],
                            lhsT=vv[:, kc * EMBED + hp * 128 + 64:
                                    kc * EMBED + hp * 128 + 128],
                            rhs=pT[:, 512:1024],
                            start=(kc == 0), stop=(kc == NKC - 1),
                            tile_position=(0, 64))

                    if qb == 0 and hp + 1 < NHP:
                        k_gather(hp + 1)
                    if hp == 1 and qb == 0:
                        v_gather(1)

                    # softmax denominators: ones^T @ acc -> [1, 512] per head
                    sums = ps_sm.tile([128, 512], F32, tag="small")
                    nc.tensor.matmul(sums[0:1, :], lhsT=ones[:],
                                     rhs=acc[:, 0:512], start=True, stop=True)
                    nc.tensor.matmul(sums[32:33, :], lhsT=ones[:],
                                     rhs=acc[:, 512:1024], start=True,
                                     stop=True, tile_position=(0, 32))
                    # copy the two sum-rows (partitions 0 and 32) to SBUF in
                    # one strided DVE op, broadcast raw sums to 128
                    # partitions with a rank-1 matmul, then one fast
                    # reciprocal over the broadcast tile
                    sums_sb = misc.tile([1, 1024], F16, tag="sums_sb")
                    with nc.allow_low_precision(
                            reason="softmax denominators, fp16 ample"):
                        nc.vector.tensor_copy(
                            sums_sb[:, 0:512], sums[0:1, 0:512])
                        nc.vector.tensor_copy(
                            sums_sb[:, 512:1024], sums[32:33, 0:512])
                    bc = ps_sm.tile([128, 512], F32, tag="small")
                    nc.tensor.matmul(bc[:], lhsT=sel[:, 0:128],
                                     rhs=sums_sb[:, 0:512], start=True,
                                     stop=False)
                    nc.tensor.matmul(bc[:], lhsT=sel[:, 128:256],
                                     rhs=sums_sb[:, 512:1024], start=False,
                                     stop=True)
                    bc_sb = misc.tile([128, 512], F32, tag="bc_sb")
                    nc.vector.reciprocal_approx_fast(bc_sb[:], bc[:])
                    nc.vector.tensor_mul(
                        aT[:, hp * TQ + qb * 512: hp * TQ + qb * 512 + 512],
                        pv[:], bc_sb[:])

            # ---- remaining out projection (qb1 token blocks) ----
            for tb in range(TQ // 256, TQ // 128):
                for ob in range(2):
                    o_proj_unit(tb, ob)

    nc.compile()
    return nc


def _get_program():
    global _PROGRAM
    if _PROGRAM is None:
        _PROGRAM = _build_program()
    return _PROGRAM


def _make_in_maps(x, q_w, q_b, k_w, k_b, v_w, v_b, o_w, o_b):
    f16 = np.float16
    # softmax scale folded into the Q projection
    wqT = np.ascontiguousarray((q_w.astype(np.float32).T / 8.0)).astype(f16)
    wkT = np.ascontiguousarray(k_w.astype(np.float32).T).astype(f16)
    wvT = np.ascontiguousarray(v_w.astype(np.float32).T).astype(f16)
    woT = np.ascontiguousarray(o_w.astype(np.float32).T).astype(f16)
    qb = np.ascontiguousarray(
        (q_b.astype(np.float32) / 8.0).reshape(NIC, 128).T)
    kb = np.ascontiguousarray(k_b.astype(np.float32).reshape(NIC, 128).T)
    vbb = np.broadcast_to(v_b.astype(np.float32), (128, EMBED)).astype(f16)
    vbb = np.ascontiguousarray(vbb)
    obb = np.ascontiguousarray(
        np.broadcast_to(o_b.astype(np.float32), (128, EMBED)))
    ones = np.ones((128, 1), f16)
    sel = np.zeros((1, 256), f16)
    sel[0, 0:64] = 1.0
    sel[0, 192:256] = 1.0
    in_maps = []
    for c in range(NCORES):
        b, qh = c // 2, c % 2
        xb = x[b].astype(np.float32)  # [T, EMBED]
        if qh == 0:
            xp = xb
        else:
            # query half first; K/V order is irrelevant (softmax sums over k)
            xp = np.concatenate([xb[TQ:], xb[:TQ]], axis=0)
        xT = np.ascontiguousarray(xp.T).astype(f16)
        in_maps.append({
            "xT": xT, "wqT": wqT, "wkT": wkT, "wvT": wvT, "woT": woT,
            "qb": qb, "kb": kb, "vbb": vbb, "obb": obb,
            "ones": ones, "sel": sel,
        })
    return in_maps


def kernel(x, mask, q_w, q_b, k_w, k_b, v_w, v_b, o_w, o_b):
    from concourse.bass_utils import run_bass_kernel_spmd

    nc = _get_program()
    x = np.asarray(x)
    in_maps = _make_in_maps(np.asarray(x), np.asarray(q_w), np.asarray(q_b),
                            np.asarray(k_w), np.asarray(k_b),
                            np.asarray(v_w), np.asarray(v_b),
                            np.asarray(o_w), np.asarray(o_b))
    res = run_bass_kernel_spmd(nc, in_maps, list(range(NCORES)))
    out = np.empty((B, T, EMBED), np.float32)
    for c in range(NCORES):
        b, qh = c // 2, c % 2
        out[b, qh * TQ:(qh + 1) * TQ, :] = res.results[c]["y"]
    return out


# revision 21
# speedup vs baseline: 1.0135x; 1.0135x over previous
"""Multi-head self-attention on 8 Trainium2 NeuronCores (Bass/Tile).

Problem: x[4, 2048, 1024], 16 heads x 64 dim, fused QKV/attention/out-proj.

Sharding (no collectives): core c handles batch b = c//2 and query-half
qh = c%2 (1024 queries), all 16 heads. K/V are computed for the full 2048
tokens of batch b (2x redundancy within a batch pair); outputs are disjoint
[1024, 1024] slices that the host concatenates.

On-chip layout (all fp16 operands, fp32 PSUM accumulation):
  - x^T [1024 in, 2048 tok] per batch, query-half tokens permuted first
  - Q^T/K^T proj: out[feat 128, tok] tiles (feature-major => heads land on
    partitions, d=64), softmax scale 1/8 folded into wq/qb on host
  - scores^T [k 128, q 512] via row-packed head-pair matmuls (d=64 each)
  - exp on ScalarE straight out of PSUM (no max subtraction: |s|/8 <~ 3)
  - P^T accumulated across k-chunks on DVE (fp16), row-sums via ones-matmul,
    reciprocal broadcast back to 128 partitions via a rank-1 matmul
  - PV col-packed per head pair -> A^T [128 feat, q], divided during PSUM
    evacuation
  - out-proj contracts the 8 A^T pair-chunks, bias added on DVE, fp32 out
"""

import numpy as np

EMBED = 1024
NH = 16
D = 64
B = 4
T = 2048
TQ = 1024  # queries per core
NCORES = 8
NIC = EMBED // 128  # 8 contraction chunks
NHP = NH // 2  # 8 head pairs

_PROGRAM = None


def _build_program():
    import concourse.bass as bass
    import concourse.mybir as mybir
    import concourse.tile as tile
    from concourse import bacc

    F16 = mybir.dt.float16
    F32 = mybir.dt.float32
    AF = mybir.ActivationFunctionType

    nc = bacc.Bacc("TRN2", target_bir_lowering=False, debug=False,
                   num_devices=NCORES)

    xT_d = nc.dram_tensor("xT", [EMBED, TQ], F16, kind="ExternalInput").ap()
    wq_d = nc.dram_tensor("wqT", [EMBED, EMBED], F16, kind="ExternalInput").ap()
    wk_d = nc.dram_tensor("wkT", [EMBED, EMBED], F16, kind="ExternalInput").ap()
    wv_d = nc.dram_tensor("wvT", [EMBED, EMBED], F16, kind="ExternalInput").ap()
    wo_d = nc.dram_tensor("woT", [EMBED, EMBED], F16, kind="ExternalInput").ap()
    qb_d = nc.dram_tensor("qb", [128, NIC], F32, kind="ExternalInput").ap()
    kb_d = nc.dram_tensor("kb", [128, NIC], F32, kind="ExternalInput").ap()
    vbb_d = nc.dram_tensor("vbb", [128, EMBED], F16, kind="ExternalInput").ap()
    obb_d = nc.dram_tensor("obb", [128, EMBED], F32, kind="ExternalInput").ap()
    ones_d = nc.dram_tensor("ones", [128, 1], F16, kind="ExternalInput").ap()
    sel_d = nc.dram_tensor("sel", [1, 256], F16, kind="ExternalInput").ap()
    y_d = nc.dram_tensor("y", [TQ, EMBED], F32, kind="ExternalOutput").ap()

    # pair-exchange staging: each core computes K/V for its own 1024 tokens
    # and AllGathers with its batch partner (replica groups of 2)
    PAIRS = [[2 * g, 2 * g + 1] for g in range(NCORES // 2)]
    kxb_d = [nc.dram_tensor(f"kxb{h}", [128, TQ], F16).ap() for h in range(NHP)]
    kg_d = [nc.dram_tensor(f"kg{h}", [2, 128, TQ], F16).ap()
            for h in range(NHP)]
    vxb_d = [nc.dram_tensor(f"vxb{o}", [TQ, 512], F16).ap() for o in range(2)]
    vg_d = [nc.dram_tensor(f"vg{o}", [2, TQ, 512], F16).ap() for o in range(2)]

    xT_r = xT_d.rearrange("(c p) t -> c p t", p=128)
    wq_r = wq_d.rearrange("(c p) o -> c p o", p=128)
    wk_r = wk_d.rearrange("(c p) o -> c p o", p=128)
    wv_r = wv_d.rearrange("(c p) o -> c p o", p=128)
    wo_r = wo_d.rearrange("(c p) o -> c p o", p=128)
    y_r = y_d.rearrange("(tb p) o -> tb p o", p=128)

    NKC = T // 128       # 16 key chunks
    NQB = TQ // 512      # 2 query blocks
    NTB = T // 128       # 16 token blocks for V
    NVO = 2              # V out-feature 512-blocks

    with tile.TileContext(nc) as tc:
        from contextlib import ExitStack
        with ExitStack() as ctx:
            cst = ctx.enter_context(tc.tile_pool(name="cst", bufs=1))
            big = ctx.enter_context(tc.tile_pool(name="big", bufs=1))
            wqk = ctx.enter_context(tc.tile_pool(name="wqk", bufs=2))
            qkp = ctx.enter_context(tc.tile_pool(name="qkp", bufs=2))
            pTp = ctx.enter_context(tc.tile_pool(name="pTp", bufs=14))
            accp = ctx.enter_context(tc.tile_pool(name="accp", bufs=2))
            misc = ctx.enter_context(tc.tile_pool(name="misc", bufs=2))
            outp = ctx.enter_context(tc.tile_pool(name="outp", bufs=3))
            ps_st = ctx.enter_context(
                tc.tile_pool(name="ps_st", bufs=2, space="PSUM"))
            ps_pv = ctx.enter_context(
                tc.tile_pool(name="ps_pv", bufs=2, space="PSUM"))
            ps_sm = ctx.enter_context(
                tc.tile_pool(name="ps_sm", bufs=2, space="PSUM"))

            # ---- persistent tiles ----
            xT = big.tile([128, NIC * TQ], F16, tag="xT")         # 16KB/par
            wv = big.tile([128, NIC * EMBED], F16, tag="wv")      # 16KB
            wo = big.tile([128, NIC * EMBED], F16, tag="wo")      # 16KB
            vv = big.tile([128, NTB * EMBED], F16, tag="vv")      # 32KB
            aT = big.tile([128, NHP * TQ], F16, tag="aT")         # 16KB
            qb_sb = cst.tile([128, NIC], F32, tag="qb")
            kb_sb = cst.tile([128, NIC], F32, tag="kb")
            vbb = cst.tile([128, EMBED], F16, tag="vbb")
            obb = cst.tile([128, EMBED], F32, tag="obb")
            ones = cst.tile([128, 1], F16, tag="ones")
            sel = cst.tile([1, 256], F16, tag="sel")

            nc.sync.dma_start(qb_sb[:], qb_d[:])
            nc.sync.dma_start(kb_sb[:], kb_d[:])
            nc.sync.dma_start(vbb[:], vbb_d[:])
            nc.sync.dma_start(obb[:], obb_d[:])
            nc.sync.dma_start(ones[:], ones_d[:])
            nc.sync.dma_start(sel[:], sel_d[:])
            for c in range(NIC):
                nc.sync.dma_start(xT[:, c * TQ:(c + 1) * TQ], xT_r[c])
                nc.sync.dma_start(wv[:, c * EMBED:(c + 1) * EMBED], wv_r[c])

            def v_proj_tb(ob, tb):
                # V[tok, feat] for OWN token block tb (<8), feat block ob*512
                ps = ps_sm.tile([128, 512], F32, tag="small")
                for c in range(NIC):
                    nc.tensor.matmul(
                        ps[:],
                        lhsT=xT[:, c * TQ + tb * 128: c * TQ + tb * 128 + 128],
                        rhs=wv[:, c * EMBED + ob * 512: c * EMBED + ob * 512 + 512],
                        start=(c == 0), stop=(c == NIC - 1))
                vtmp = outp.tile([128, 512], F16, tag="vtmp")
                nc.vector.tensor_add(vtmp[:], ps[:],
                                     vbb[:, ob * 512:(ob + 1) * 512])
                nc.sync.dma_start(
                    vxb_d[ob][tb * 128:(tb + 1) * 128, :], vtmp[:])

            def v_gather(ob):
                nc.gpsimd.collective_compute(
                    "AllGather", mybir.AluOpType.bypass,
                    ins=[vxb_d[ob][:]], outs=[vg_d[ob][:]],
                    replica_groups=PAIRS)
                for g in range(2):
                    for tb in range(TQ // 128):
                        gt = g * (TQ // 128) + tb
                        nc.sync.dma_start(
                            vv[:, gt * EMBED + ob * 512:
                               gt * EMBED + ob * 512 + 512],
                            vg_d[ob][g, tb * 128:(tb + 1) * 128, :])

            # ---- per-head-pair K/Q projection, emitted one pair AHEAD,
            # interleaved into the previous pair's attention loop so the
            # scalar engine never drains between pairs ----
            kq = {}

            def alloc_kq(hp):
                wq_sb = wqk.tile([128, NIC * 128], F16, tag="wq")
                wk_sb = wqk.tile([128, NIC * 128], F16, tag="wk")
                for c in range(NIC):
                    nc.sync.dma_start(
                        wq_sb[:, c * 128:(c + 1) * 128],
                        wq_r[c][:, hp * 128:(hp + 1) * 128])
                    nc.sync.dma_start(
                        wk_sb[:, c * 128:(c + 1) * 128],
                        wk_r[c][:, hp * 128:(hp + 1) * 128])
                kT = qkp.tile([128, T], F16, tag="kT")
                qT = qkp.tile([128, TQ], F16, tag="qT")
                kTh = qkp.tile([128, TQ], F16, tag="kTh")
                kq[hp] = (wq_sb, wk_sb, kT, qT, kTh)

            def k_proj_tb(hp, tb):
                # K^T for OWN tokens only (tb < 2), then pair-exchange
                wq_sb, wk_sb, kT, qT, kTh = kq[hp]
                ps = ps_sm.tile([128, 512], F32, tag="small")
                for c in range(NIC):
                    nc.tensor.matmul(
                        ps[:], lhsT=wk_sb[:, c * 128:(c + 1) * 128],
                        rhs=xT[:, c * TQ + tb * 512: c * TQ + tb * 512 + 512],
                        start=(c == 0), stop=(c == NIC - 1))
                nc.vector.tensor_scalar_add(
                    kTh[:, tb * 512:(tb + 1) * 512], ps[:], kb_sb[:, hp:hp + 1])

            def k_gather(hp):
                _, _, kT, _, kTh = kq[hp]
                nc.sync.dma_start(kxb_d[hp][:], kTh[:])
                nc.gpsimd.collective_compute(
                    "AllGather", mybir.AluOpType.bypass,
                    ins=[kxb_d[hp][:]], outs=[kg_d[hp][:]],
                    replica_groups=PAIRS)
                nc.sync.dma_start(kT[:, 0:TQ], kg_d[hp][0])
                nc.sync.dma_start(kT[:, TQ:T], kg_d[hp][1])

            def q_proj_tb(hp, tb):
                wq_sb, wk_sb, kT, qT, kTh = kq[hp]
                ps = ps_sm.tile([128, 512], F32, tag="small")
                for c in range(NIC):
                    nc.tensor.matmul(
                        ps[:], lhsT=wq_sb[:, c * 128:(c + 1) * 128],
                        rhs=xT[:, c * TQ + tb * 512: c * TQ + tb * 512 + 512],
                        start=(c == 0), stop=(c == NIC - 1))
                nc.vector.tensor_scalar_add(
                    qT[:, tb * 512:(tb + 1) * 512], ps[:], qb_sb[:, hp:hp + 1])

            def o_proj_unit(tb, ob):
                ps = ps_sm.tile([128, 512], F32, tag="small")
                for f in range(NHP):
                    nc.tensor.matmul(
                        ps[:],
                        lhsT=aT[:, f * TQ + tb * 128: f * TQ + tb * 128 + 128],
                        rhs=wo[:, f * EMBED + ob * 512:
                               f * EMBED + ob * 512 + 512],
                        start=(f == 0), stop=(f == NHP - 1))
                out_sb = outp.tile([128, 512], F32, tag="out")
                nc.vector.tensor_add(out_sb[:], ps[:],
                                     obb[:, ob * 512:(ob + 1) * 512])
                nc.sync.dma_start(y_r[tb][:, ob * 512:(ob + 1) * 512],
                                  out_sb[:])

            alloc_kq(0)
            k_proj_tb(0, 0)
            k_proj_tb(0, 1)
            k_gather(0)
            q_proj_tb(0, 0)
            q_proj_tb(0, 1)
            for tb in range(TQ // 128):
                v_proj_tb(0, tb)
            v_gather(0)
            for c in range(NIC):
                nc.sync.dma_start(wo[:, c * EMBED:(c + 1) * EMBED], wo_r[c])

            pending_tail = [None]

            def flush_tail():
                if pending_tail[0] is None:
                    return
                hp_, qb_, pv_, acc_ = pending_tail[0]
                pending_tail[0] = None
                # softmax denominators: ones^T @ acc -> [1, 512] per head
                sums = ps_sm.tile([128, 512], F32, tag="small")
                nc.tensor.matmul(sums[0:1, :], lhsT=ones[:],
                                 rhs=acc_[:, 0:512], start=True, stop=True)
                nc.tensor.matmul(sums[32:33, :], lhsT=ones[:],
                                 rhs=acc_[:, 512:1024], start=True,
                                 stop=True, tile_position=(0, 32))
                sums_sb = misc.tile([1, 1024], F16, tag="sums_sb")
                with nc.allow_low_precision(
                        reason="softmax denominators, fp16 ample"):
                    nc.vector.tensor_copy(
                        sums_sb[:, 0:512], sums[0:1, 0:512])
                    nc.vector.tensor_copy(
                        sums_sb[:, 512:1024], sums[32:33, 0:512])
                bc = ps_sm.tile([128, 512], F32, tag="small")
                nc.tensor.matmul(bc[:], lhsT=sel[:, 0:128],
                                 rhs=sums_sb[:, 0:512], start=True,
                                 stop=False)
                nc.tensor.matmul(bc[:], lhsT=sel[:, 128:256],
                                 rhs=sums_sb[:, 512:1024], start=False,
                                 stop=True)
                bc_sb = misc.tile([128, 512], F32, tag="bc_sb")
                nc.vector.reciprocal_approx_fast(bc_sb[:], bc[:])
                nc.vector.tensor_mul(
                    aT[:, hp_ * TQ + qb_ * 512: hp_ * TQ + qb_ * 512 + 512],
                    pv_[:], bc_sb[:])

            for hp in range(NHP):
                _, _, kT, qT, _ = kq[hp]

                for qb in range(NQB):
                    if hp + 1 < NHP and qb == 0:
                        alloc_kq(hp + 1)
                    # interleave units: next-pair K/Q proj, the second V
                    # feature-block under hp1, and the first half of the
                    # out-projection under hp7 qb1
                    units = {}
                    if hp + 1 < NHP:
                        if qb == 0:
                            units = {4: (k_proj_tb, hp + 1, 0),
                                     12: (k_proj_tb, hp + 1, 1)}
                        else:
                            units = {2: (q_proj_tb, hp + 1, 0),
                                     10: (q_proj_tb, hp + 1, 1)}
                    elif qb == 1:
                        units = {k: (o_proj_unit, u // 2, u % 2)
                                 for u, k in enumerate(
                                     [3, 4, 5, 6, 7, 9, 11, 13])}
                    if hp == 0 and qb == 1:
                        for u in range(TQ // 128):
                            units[2 * u + 1] = (v_proj_tb, 1, u)
                    pv = ps_pv.tile([128, 512], F32, tag="pv")
                    acc = accp.tile([128, 1024], F16, tag="acc")
                    prev_pT = None
                    for kc in range(NKC):
                        if kc == 2:
                            flush_tail()
                        if kc in units:
                            fn, a0, a1 = units[kc]
                            fn(a0, a1)
                        st = ps_st.tile([128, 1024], F32, tag="st")
                        nc.tensor.matmul(
                            st[:, 0:512],
                            lhsT=kT[0:64, kc * 128:(kc + 1) * 128],
                            rhs=qT[0:64, qb * 512:(qb + 1) * 512],
                            start=True, stop=True)
                        nc.tensor.matmul(
                            st[:, 512:1024],
                            lhsT=kT[64:128, kc * 128:(kc + 1) * 128],
                            rhs=qT[64:128, qb * 512:(qb + 1) * 512],
                            start=True, stop=True, tile_position=(64, 0))
                        pT = pTp.tile([128, 1024], F16, tag="pT")
                        nc.scalar.activation(pT[:], st[:], AF.Exp)
                        with nc.allow_low_precision(
                                reason="fp16 softmax partial-sum accumulate"):
                            if kc == 1:
                                nc.vector.tensor_add(
                                    acc[:], prev_pT[:], pT[:])
                            elif kc > 1:
                                nc.vector.tensor_add(acc[:], acc[:], pT[:])
                        prev_pT = pT
                        nc.tensor.matmul(
                            pv[0:64, :],
                            lhsT=vv[:, kc * EMBED + hp * 128:
                                    kc * EMBED + hp * 128 + 64],
                            rhs=pT[:, 0:512],
                            start=(kc == 0), stop=(kc == NKC - 1))
                        nc.tensor.matmul(
                            pv[64:128, :],
                            lhsT=vv[:, kc * EMBED + hp * 128 + 64:
                                    kc * EMBED + hp * 128 + 128],
                            rhs=pT[:, 512:1024],
                            start=(kc == 0), stop=(kc == NKC - 1),
                            tile_position=(0, 64))

                    if qb == 0 and hp + 1 < NHP:
                        k_gather(hp + 1)
                    if hp == 0 and qb == 1:
                        v_gather(1)

                    # defer this iteration's softmax tail so the next
                    # iteration's first scores/exp keep ScalarE fed
                    pending_tail[0] = (hp, qb, pv, acc)

            flush_tail()

            # ---- remaining out projection (qb1 token blocks) ----
            for tb in range(TQ // 256, TQ // 128):
                for ob in range(2):
                    o_proj_unit(tb, ob)

    nc.compile()
    return nc


def _get_program():
    global _PROGRAM
    if _PROGRAM is None:
        _PROGRAM = _build_program()
    return _PROGRAM


def _make_in_maps(x, q_w, q_b, k_w, k_b, v_w, v_b, o_w, o_b):
    f16 = np.float16
    # softmax scale folded into the Q projection
    wqT = np.ascontiguousarray((q_w.astype(np.float32).T / 8.0)).astype(f16)
    wkT = np.ascontiguousarray(k_w.astype(np.float32).T).astype(f16)
    wvT = np.ascontiguousarray(v_w.astype(np.float32).T).astype(f16)
    woT = np.ascontiguousarray(o_w.astype(np.float32).T).astype(f16)
    qb = np.ascontiguousarray(
        (q_b.astype(np.float32) / 8.0).reshape(NIC, 128).T)
    kb = np.ascontiguousarray(k_b.astype(np.float32).reshape(NIC, 128).T)
    vbb = np.broadcast_to(v_b.astype(np.float32), (128, EMBED)).astype(f16)
    vbb = np.ascontiguousarray(vbb)
    obb = np.ascontiguousarray(
        np.broadcast_to(o_b.astype(np.float32), (128, EMBED)))
    ones = np.ones((128, 1), f16)
    sel = np.zeros((1, 256), f16)
    sel[0, 0:64] = 1.0
    sel[0, 192:256] = 1.0
    in_maps = []
    for c in range(NCORES):
        b, qh = c // 2, c % 2
        # own-token slab only; the partner's K/V arrive via the pair
        # AllGather on device (k ordering is irrelevant to softmax)
        xT = np.ascontiguousarray(
            x[b, qh * TQ:(qh + 1) * TQ].astype(np.float32).T).astype(f16)
        in_maps.append({
            "xT": xT, "wqT": wqT, "wkT": wkT, "wvT": wvT, "woT": woT,
            "qb": qb, "kb": kb, "vbb": vbb, "obb": obb,
            "ones": ones, "sel": sel,
        })
    return in_maps


def kernel(x, mask, q_w, q_b, k_w, k_b, v_w, v_b, o_w, o_b):
    from concourse.bass_utils import run_bass_kernel_spmd

    nc = _get_program()
    x = np.asarray(x)
    in_maps = _make_in_maps(np.asarray(x), np.asarray(q_w), np.asarray(q_b),
                            np.asarray(k_w), np.asarray(k_b),
                            np.asarray(v_w), np.asarray(v_b),
                            np.asarray(o_w), np.asarray(o_b))
    res = run_bass_kernel_spmd(nc, in_maps, list(range(NCORES)))
    out = np.empty((B, T, EMBED), np.float32)
    for c in range(NCORES):
        b, qh = c // 2, c % 2
        out[b, qh * TQ:(qh + 1) * TQ, :] = res.results[c]["y"]
    return out


# revision 22
# speedup vs baseline: 1.0417x; 1.0278x over previous
"""Multi-head self-attention on 8 Trainium2 NeuronCores (Bass/Tile).

Problem: x[4, 2048, 1024], 16 heads x 64 dim, fused QKV/attention/out-proj.

Sharding (no collectives): core c handles batch b = c//2 and query-half
qh = c%2 (1024 queries), all 16 heads. K/V are computed for the full 2048
tokens of batch b (2x redundancy within a batch pair); outputs are disjoint
[1024, 1024] slices that the host concatenates.

On-chip layout (all fp16 operands, fp32 PSUM accumulation):
  - x^T [1024 in, 2048 tok] per batch, query-half tokens permuted first
  - Q^T/K^T proj: out[feat 128, tok] tiles (feature-major => heads land on
    partitions, d=64), softmax scale 1/8 folded into wq/qb on host
  - scores^T [k 128, q 512] via row-packed head-pair matmuls (d=64 each)
  - exp on ScalarE straight out of PSUM (no max subtraction: |s|/8 <~ 3)
  - P^T accumulated across k-chunks on DVE (fp16), row-sums via ones-matmul,
    reciprocal broadcast back to 128 partitions via a rank-1 matmul
  - PV col-packed per head pair -> A^T [128 feat, q], divided during PSUM
    evacuation
  - out-proj contracts the 8 A^T pair-chunks, bias added on DVE, fp32 out
"""

import numpy as np

EMBED = 1024
NH = 16
D = 64
B = 4
T = 2048
TQ = 1024  # queries per core
NCORES = 8
NIC = EMBED // 128  # 8 contraction chunks
NHP = NH // 2  # 8 head pairs

_PROGRAM = None


def _build_program():
    import concourse.bass as bass
    import concourse.mybir as mybir
    import concourse.tile as tile
    from concourse import bacc

    F16 = mybir.dt.float16
    F32 = mybir.dt.float32
    AF = mybir.ActivationFunctionType

    nc = bacc.Bacc("TRN2", target_bir_lowering=False, debug=False,
                   num_devices=NCORES)

    xT_d = nc.dram_tensor("xT", [EMBED, TQ], F16, kind="ExternalInput").ap()
    wq_d = nc.dram_tensor("wqT", [EMBED, EMBED], F16, kind="ExternalInput").ap()
    wk_d = nc.dram_tensor("wkT", [EMBED, EMBED], F16, kind="ExternalInput").ap()
    wv_d = nc.dram_tensor("wvT", [EMBED, EMBED], F16, kind="ExternalInput").ap()
    wo_d = nc.dram_tensor("woT", [EMBED, EMBED], F16, kind="ExternalInput").ap()
    qb_d = nc.dram_tensor("qb", [128, NIC], F32, kind="ExternalInput").ap()
    kb_d = nc.dram_tensor("kb", [128, NIC], F32, kind="ExternalInput").ap()
    vbb_d = nc.dram_tensor("vbb", [128, EMBED], F16, kind="ExternalInput").ap()
    obb_d = nc.dram_tensor("obb", [128, EMBED], F32, kind="ExternalInput").ap()
    ones_d = nc.dram_tensor("ones", [128, 1], F16, kind="ExternalInput").ap()
    sel_d = nc.dram_tensor("sel", [1, 256], F16, kind="ExternalInput").ap()
    y_d = nc.dram_tensor("y", [TQ, EMBED], F32, kind="ExternalOutput").ap()

    # pair-exchange staging: each core computes K/V for its own 1024 tokens
    # and AllGathers with its batch partner (replica groups of 2)
    PAIRS = [[2 * g, 2 * g + 1] for g in range(NCORES // 2)]
    kxb_d = [nc.dram_tensor(f"kxb{h}", [128, TQ], F16).ap() for h in range(NHP)]
    kg_d = [nc.dram_tensor(f"kg{h}", [2, 128, TQ], F16).ap()
            for h in range(NHP)]
    vxb_d = [[nc.dram_tensor(f"vxb{o}h{h}", [512, 512], F16).ap()
              for h in range(2)] for o in range(2)]
    vg_d = [[nc.dram_tensor(f"vg{o}h{h}", [2, 512, 512], F16).ap()
             for h in range(2)] for o in range(2)]

    xT_r = xT_d.rearrange("(c p) t -> c p t", p=128)
    wq_r = wq_d.rearrange("(c p) o -> c p o", p=128)
    wk_r = wk_d.rearrange("(c p) o -> c p o", p=128)
    wv_r = wv_d.rearrange("(c p) o -> c p o", p=128)
    wo_r = wo_d.rearrange("(c p) o -> c p o", p=128)
    y_r = y_d.rearrange("(tb p) o -> tb p o", p=128)

    NKC = T // 128       # 16 key chunks
    NQB = TQ // 512      # 2 query blocks
    NTB = T // 128       # 16 token blocks for V
    NVO = 2              # V out-feature 512-blocks

    with tile.TileContext(nc) as tc:
        from contextlib import ExitStack
        with ExitStack() as ctx:
            cst = ctx.enter_context(tc.tile_pool(name="cst", bufs=1))
            big = ctx.enter_context(tc.tile_pool(name="big", bufs=1))
            wqk = ctx.enter_context(tc.tile_pool(name="wqk", bufs=2))
            qkp = ctx.enter_context(tc.tile_pool(name="qkp", bufs=2))
            pTp = ctx.enter_context(tc.tile_pool(name="pTp", bufs=14))
            accp = ctx.enter_context(tc.tile_pool(name="accp", bufs=2))
            misc = ctx.enter_context(tc.tile_pool(name="misc", bufs=2))
            outp = ctx.enter_context(tc.tile_pool(name="outp", bufs=3))
            ps_st = ctx.enter_context(
                tc.tile_pool(name="ps_st", bufs=2, space="PSUM"))
            ps_pv = ctx.enter_context(
                tc.tile_pool(name="ps_pv", bufs=2, space="PSUM"))
            ps_sm = ctx.enter_context(
                tc.tile_pool(name="ps_sm", bufs=2, space="PSUM"))

            # ---- persistent tiles ----
            xT = big.tile([128, NIC * TQ], F16, tag="xT")         # 16KB/par
            wv = big.tile([128, NIC * EMBED], F16, tag="wv")      # 16KB
            wo = big.tile([128, NIC * EMBED], F16, tag="wo")      # 16KB
            vv = big.tile([128, NTB * EMBED], F16, tag="vv")      # 32KB
            aT = big.tile([128, NHP * TQ], F16, tag="aT")         # 16KB
            qb_sb = cst.tile([128, NIC], F32, tag="qb")
            kb_sb = cst.tile([128, NIC], F32, tag="kb")
            vbb = cst.tile([128, EMBED], F16, tag="vbb")
            obb = cst.tile([128, EMBED], F32, tag="obb")
            ones = cst.tile([128, 1], F16, tag="ones")
            sel = cst.tile([1, 256], F16, tag="sel")

            for c in range(NIC):
                nc.sync.dma_start(xT[:, c * TQ:(c + 1) * TQ], xT_r[c])
            nc.sync.dma_start(qb_sb[:], qb_d[:])
            nc.sync.dma_start(kb_sb[:], kb_d[:])
            nc.sync.dma_start(ones[:], ones_d[:])
            nc.sync.dma_start(sel[:], sel_d[:])
            nc.sync.dma_start(vbb[:], vbb_d[:])
            nc.sync.dma_start(obb[:], obb_d[:])
            # warm the exp activation table while DMAs stream
            warm = misc.tile([128, 1], F16, tag="warm")
            nc.scalar.activation(warm[:], ones[:], AF.Exp)

            def v_proj_tb(ob, tb):
                # V[tok, feat] for OWN token block tb (<8), feat block ob*512
                ps = ps_sm.tile([128, 512], F32, tag="small")
                for c in range(NIC):
                    nc.tensor.matmul(
                        ps[:],
                        lhsT=xT[:, c * TQ + tb * 128: c * TQ + tb * 128 + 128],
                        rhs=wv[:, c * EMBED + ob * 512: c * EMBED + ob * 512 + 512],
                        start=(c == 0), stop=(c == NIC - 1))
                vtmp = outp.tile([128, 512], F16, tag="vtmp")
                nc.vector.tensor_add(vtmp[:], ps[:],
                                     vbb[:, ob * 512:(ob + 1) * 512])
                nc.sync.dma_start(
                    vxb_d[ob][tb // 4][(tb % 4) * 128:(tb % 4) * 128 + 128, :],
                    vtmp[:])

            def v_gather(ob, half):
                nc.gpsimd.collective_compute(
                    "AllGather", mybir.AluOpType.bypass,
                    ins=[vxb_d[ob][half][:]], outs=[vg_d[ob][half][:]],
                    replica_groups=PAIRS)
                for g in range(2):
                    for j in range(4):
                        gt = g * 8 + half * 4 + j
                        nc.sync.dma_start(
                            vv[:, gt * EMBED + ob * 512:
                               gt * EMBED + ob * 512 + 512],
                            vg_d[ob][half][g, j * 128:(j + 1) * 128, :])

            # ---- per-head-pair K/Q projection, emitted one pair AHEAD,
            # interleaved into the previous pair's attention loop so the
            # scalar engine never drains between pairs ----
            kq = {}

            def alloc_kq(hp):
                wq_sb = wqk.tile([128, NIC * 128], F16, tag="wq")
                wk_sb = wqk.tile([128, NIC * 128], F16, tag="wk")
                for c in range(NIC):
                    nc.sync.dma_start(
                        wq_sb[:, c * 128:(c + 1) * 128],
                        wq_r[c][:, hp * 128:(hp + 1) * 128])
                    nc.sync.dma_start(
                        wk_sb[:, c * 128:(c + 1) * 128],
                        wk_r[c][:, hp * 128:(hp + 1) * 128])
                kT = qkp.tile([128, T], F16, tag="kT")
                qT = qkp.tile([128, TQ], F16, tag="qT")
                kTh = qkp.tile([128, TQ], F16, tag="kTh")
                kq[hp] = (wq_sb, wk_sb, kT, qT, kTh)

            def k_proj_tb(hp, tb):
                # K^T for OWN tokens only (tb < 2), then pair-exchange
                wq_sb, wk_sb, kT, qT, kTh = kq[hp]
                ps = ps_sm.tile([128, 512], F32, tag="small")
                for c in range(NIC):
                    nc.tensor.matmul(
                        ps[:], lhsT=wk_sb[:, c * 128:(c + 1) * 128],
                        rhs=xT[:, c * TQ + tb * 512: c * TQ + tb * 512 + 512],
                        start=(c == 0), stop=(c == NIC - 1))
                nc.vector.tensor_scalar_add(
                    kTh[:, tb * 512:(tb + 1) * 512], ps[:], kb_sb[:, hp:hp + 1])

            def k_gather(hp):
                _, _, kT, _, kTh = kq[hp]
                nc.sync.dma_start(kxb_d[hp][:], kTh[:])
                nc.gpsimd.collective_compute(
                    "AllGather", mybir.AluOpType.bypass,
                    ins=[kxb_d[hp][:]], outs=[kg_d[hp][:]],
                    replica_groups=PAIRS)
                nc.sync.dma_start(kT[:, 0:TQ], kg_d[hp][0])
                nc.sync.dma_start(kT[:, TQ:T], kg_d[hp][1])

            def q_proj_tb(hp, tb):
                wq_sb, wk_sb, kT, qT, kTh = kq[hp]
                ps = ps_sm.tile([128, 512], F32, tag="small")
                for c in range(NIC):
                    nc.tensor.matmul(
                        ps[:], lhsT=wq_sb[:, c * 128:(c + 1) * 128],
                        rhs=xT[:, c * TQ + tb * 512: c * TQ + tb * 512 + 512],
                        start=(c == 0), stop=(c == NIC - 1))
                nc.vector.tensor_scalar_add(
                    qT[:, tb * 512:(tb + 1) * 512], ps[:], qb_sb[:, hp:hp + 1])

            def o_proj_unit(tb, ob):
                ps = ps_sm.tile([128, 512], F32, tag="small")
                for f in range(NHP):
                    nc.tensor.matmul(
                        ps[:],
                        lhsT=aT[:, f * TQ + tb * 128: f * TQ + tb * 128 + 128],
                        rhs=wo[:, f * EMBED + ob * 512:
                               f * EMBED + ob * 512 + 512],
                        start=(f == 0), stop=(f == NHP - 1))
                out_sb = outp.tile([128, 512], F32, tag="out")
                nc.vector.tensor_add(out_sb[:], ps[:],
                                     obb[:, ob * 512:(ob + 1) * 512])
                nc.sync.dma_start(y_r[tb][:, ob * 512:(ob + 1) * 512],
                                  out_sb[:])

            alloc_kq(0)
            k_proj_tb(0, 0)
            k_proj_tb(0, 1)
            k_gather(0)
            for c in range(NIC):
                nc.sync.dma_start(wv[:, c * EMBED:(c + 1) * EMBED], wv_r[c])
            q_proj_tb(0, 0)
            q_proj_tb(0, 1)
            for tb in range(4):
                v_proj_tb(0, tb)
            v_gather(0, 0)
            for tb in range(4, 8):
                v_proj_tb(0, tb)
            v_gather(0, 1)
            for c in range(NIC):
                nc.sync.dma_start(wo[:, c * EMBED:(c + 1) * EMBED], wo_r[c])

            pending_tail = [None]

            def flush_tail():
                if pending_tail[0] is None:
                    return
                hp_, qb_, pv_, acc_ = pending_tail[0]
                pending_tail[0] = None
                # softmax denominators: ones^T @ acc -> [1, 512] per head
                sums = ps_sm.tile([128, 512], F32, tag="small")
                nc.tensor.matmul(sums[0:1, :], lhsT=ones[:],
                                 rhs=acc_[:, 0:512], start=True, stop=True)
                nc.tensor.matmul(sums[32:33, :], lhsT=ones[:],
                                 rhs=acc_[:, 512:1024], start=True,
                                 stop=True, tile_position=(0, 32))
                sums_sb = misc.tile([1, 1024], F16, tag="sums_sb")
                with nc.allow_low_precision(
                        reason="softmax denominators, fp16 ample"):
                    nc.vector.tensor_copy(
                        sums_sb[:, 0:512], sums[0:1, 0:512])
                    nc.vector.tensor_copy(
                        sums_sb[:, 512:1024], sums[32:33, 0:512])
                bc = ps_sm.tile([128, 512], F32, tag="small")
                nc.tensor.matmul(bc[:], lhsT=sel[:, 0:128],
                                 rhs=sums_sb[:, 0:512], start=True,
                                 stop=False)
                nc.tensor.matmul(bc[:], lhsT=sel[:, 128:256],
                                 rhs=sums_sb[:, 512:1024], start=False,
                                 stop=True)
                bc_sb = misc.tile([128, 512], F32, tag="bc_sb")
                nc.vector.reciprocal_approx_fast(bc_sb[:], bc[:])
                nc.vector.tensor_mul(
                    aT[:, hp_ * TQ + qb_ * 512: hp_ * TQ + qb_ * 512 + 512],
                    pv_[:], bc_sb[:])

            for hp in range(NHP):
                _, _, kT, qT, _ = kq[hp]

                for qb in range(NQB):
                    if hp + 1 < NHP and qb == 0:
                        alloc_kq(hp + 1)
                    # interleave units: next-pair K/Q proj, the second V
                    # feature-block under hp1, and the first half of the
                    # out-projection under hp7 qb1
                    units = {}
                    if hp + 1 < NHP:
                        if qb == 0:
                            units = {1: (k_proj_tb, hp + 1, 0),
                                     5: (k_proj_tb, hp + 1, 1),
                                     9: (k_gather, hp + 1, None)}
                        else:
                            units = {2: (q_proj_tb, hp + 1, 0),
                                     10: (q_proj_tb, hp + 1, 1)}
                    elif qb == 1:
                        units = {k: (o_proj_unit, u // 2, u % 2)
                                 for u, k in enumerate(
                                     [3, 4, 5, 6, 7, 9, 11, 13])}
                    if hp == 1:
                        if qb == 0:
                            units[3] = (v_proj_tb, 1, 0)
                            units[7] = (v_proj_tb, 1, 1)
                            units[11] = (v_proj_tb, 1, 2)
                            units[13] = (v_proj_tb, 1, 3)
                            units[15] = (v_gather, 1, 0)
                        else:
                            units[4] = (v_proj_tb, 1, 4)
                            units[6] = (v_proj_tb, 1, 5)
                            units[8] = (v_proj_tb, 1, 6)
                            units[12] = (v_proj_tb, 1, 7)
                            units[14] = (v_gather, 1, 1)
                    pv = ps_pv.tile([128, 512], F32, tag="pv")
                    acc = accp.tile([128, 1024], F16, tag="acc")
                    prev_pT = None
                    for kc in range(NKC):
                        if kc == 2:
                            flush_tail()
                        if kc in units:
                            fn, a0, a1 = units[kc]
                            if a1 is None:
                                fn(a0)
                            else:
                                fn(a0, a1)
                        st = ps_st.tile([128, 1024], F32, tag="st")
                        nc.tensor.matmul(
                            st[:, 0:512],
                            lhsT=kT[0:64, kc * 128:(kc + 1) * 128],
                            rhs=qT[0:64, qb * 512:(qb + 1) * 512],
                            start=True, stop=True)
                        nc.tensor.matmul(
                            st[:, 512:1024],
                            lhsT=kT[64:128, kc * 128:(kc + 1) * 128],
                            rhs=qT[64:128, qb * 512:(qb + 1) * 512],
                            start=True, stop=True, tile_position=(64, 0))
                        pT = pTp.tile([128, 1024], F16, tag="pT")
                        nc.scalar.activation(pT[:], st[:], AF.Exp)
                        with nc.allow_low_precision(
                                reason="fp16 softmax partial-sum accumulate"):
                            if kc == 1:
                                nc.vector.tensor_add(
                                    acc[:], prev_pT[:], pT[:])
                            elif kc > 1:
                                nc.vector.tensor_add(acc[:], acc[:], pT[:])
                        prev_pT = pT
                        nc.tensor.matmul(
                            pv[0:64, :],
                            lhsT=vv[:, kc * EMBED + hp * 128:
                                    kc * EMBED + hp * 128 + 64],
                            rhs=pT[:, 0:512],
                            start=(kc == 0), stop=(kc == NKC - 1))
                        nc.tensor.matmul(
                            pv[64:128, :],
                            lhsT=vv[:, kc * EMBED + hp * 128 + 64:
                                    kc * EMBED + hp * 128 + 128],
                            rhs=pT[:, 512:1024],
                            start=(kc == 0), stop=(kc == NKC - 1),
                            tile_position=(0, 64))

                    # defer this iteration's softmax tail so the next
                    # iteration's first scores/exp keep ScalarE fed
                    pending_tail[0] = (hp, qb, pv, acc)

            flush_tail()

            # ---- remaining out projection (qb1 token blocks) ----
            for tb in range(TQ // 256, TQ // 128):
                for ob in range(2):
                    o_proj_unit(tb, ob)

    nc.compile()
    return nc


def _get_program():
    global _PROGRAM
    if _PROGRAM is None:
        _PROGRAM = _build_program()
    return _PROGRAM


def _make_in_maps(x, q_w, q_b, k_w, k_b, v_w, v_b, o_w, o_b):
    f16 = np.float16
    # softmax scale folded into the Q projection
    wqT = np.ascontiguousarray((q_w.astype(np.float32).T / 8.0)).astype(f16)
    wkT = np.ascontiguousarray(k_w.astype(np.float32).T).astype(f16)
    wvT = np.ascontiguousarray(v_w.astype(np.float32).T).astype(f16)
    woT = np.ascontiguousarray(o_w.astype(np.float32).T).astype(f16)
    qb = np.ascontiguousarray(
        (q_b.astype(np.float32) / 8.0).reshape(NIC, 128).T)
    kb = np.ascontiguousarray(k_b.astype(np.float32).reshape(NIC, 128).T)
    vbb = np.broadcast_to(v_b.astype(np.float32), (128, EMBED)).astype(f16)
    vbb = np.ascontiguousarray(vbb)
    obb = np.ascontiguousarray(
        np.broadcast_to(o_b.astype(np.float32), (128, EMBED)))
    ones = np.ones((128, 1), f16)
    sel = np.zeros((1, 256), f16)
    sel[0, 0:64] = 1.0
    sel[0, 192:256] = 1.0
    in_maps = []
    for c in range(NCORES):
        b, qh = c // 2, c % 2
        # own-token slab only; the partner's K/V arrive via the pair
        # AllGather on device (k ordering is irrelevant to softmax)
        xT = np.ascontiguousarray(
            x[b, qh * TQ:(qh + 1) * TQ].astype(np.float32).T).astype(f16)
        in_maps.append({
            "xT": xT, "wqT": wqT, "wkT": wkT, "wvT": wvT, "woT": woT,
            "qb": qb, "kb": kb, "vbb": vbb, "obb": obb,
            "ones": ones, "sel": sel,
        })
    return in_maps


def kernel(x, mask, q_w, q_b, k_w, k_b, v_w, v_b, o_w, o_b):
    from concourse.bass_utils import run_bass_kernel_spmd

    nc = _get_program()
    x = np.asarray(x)
    in_maps = _make_in_maps(np.asarray(x), np.asarray(q_w), np.asarray(q_b),
                            np.asarray(k_w), np.asarray(k_b),
                            np.asarray(v_w), np.asarray(v_b),
                            np.asarray(o_w), np.asarray(o_b))
    res = run_bass_kernel_spmd(nc, in_maps, list(range(NCORES)))
    out = np.empty((B, T, EMBED), np.float32)
    for c in range(NCORES):
        b, qh = c // 2, c % 2
        out[b, qh * TQ:(qh + 1) * TQ, :] = res.results[c]["y"]
    return out


# revision 23
# speedup vs baseline: 1.0662x; 1.0235x over previous
"""Multi-head self-attention on 8 Trainium2 NeuronCores (Bass/Tile).

Problem: x[4, 2048, 1024], 16 heads x 64 dim, fused QKV/attention/out-proj.

Sharding (no collectives): core c handles batch b = c//2 and query-half
qh = c%2 (1024 queries), all 16 heads. K/V are computed for the full 2048
tokens of batch b (2x redundancy within a batch pair); outputs are disjoint
[1024, 1024] slices that the host concatenates.

On-chip layout (all fp16 operands, fp32 PSUM accumulation):
  - x^T [1024 in, 2048 tok] per batch, query-half tokens permuted first
  - Q^T/K^T proj: out[feat 128, tok] tiles (feature-major => heads land on
    partitions, d=64), softmax scale 1/8 folded into wq/qb on host
  - scores^T [k 128, q 512] via row-packed head-pair matmuls (d=64 each)
  - exp on ScalarE straight out of PSUM (no max subtraction: |s|/8 <~ 3)
  - P^T accumulated across k-chunks on DVE (fp16), row-sums via ones-matmul,
    reciprocal broadcast back to 128 partitions via a rank-1 matmul
  - PV col-packed per head pair -> A^T [128 feat, q], divided during PSUM
    evacuation
  - out-proj contracts the 8 A^T pair-chunks, bias added on DVE, fp32 out
"""

import numpy as np

EMBED = 1024
NH = 16
D = 64
B = 4
T = 2048
TQ = 1024  # queries per core
NCORES = 8
NIC = EMBED // 128  # 8 contraction chunks
NHP = NH // 2  # 8 head pairs

_PROGRAM = None


def _build_program():
    import concourse.bass as bass
    import concourse.mybir as mybir
    import concourse.tile as tile
    from concourse import bacc

    F16 = mybir.dt.float16
    F32 = mybir.dt.float32
    AF = mybir.ActivationFunctionType

    nc = bacc.Bacc("TRN2", target_bir_lowering=False, debug=False,
                   num_devices=NCORES)

    xT_d = nc.dram_tensor("xT", [EMBED, TQ], F16, kind="ExternalInput").ap()
    wq_d = nc.dram_tensor("wqT", [EMBED, EMBED], F16, kind="ExternalInput").ap()
    wk_d = nc.dram_tensor("wkT", [EMBED, EMBED], F16, kind="ExternalInput").ap()
    wv_d = nc.dram_tensor("wvT", [EMBED, EMBED], F16, kind="ExternalInput").ap()
    wo_d = nc.dram_tensor("woT", [EMBED, EMBED], F16, kind="ExternalInput").ap()
    qb_d = nc.dram_tensor("qb", [128, NIC], F32, kind="ExternalInput").ap()
    kb_d = nc.dram_tensor("kb", [128, NIC], F32, kind="ExternalInput").ap()
    vbb_d = nc.dram_tensor("vbb", [128, EMBED], F16, kind="ExternalInput").ap()
    obb_d = nc.dram_tensor("obb", [128, EMBED], F32, kind="ExternalInput").ap()
    ones_d = nc.dram_tensor("ones", [128, 1], F16, kind="ExternalInput").ap()
    sel_d = nc.dram_tensor("sel", [1, 256], F16, kind="ExternalInput").ap()
    y_d = nc.dram_tensor("y", [TQ, EMBED], F32, kind="ExternalOutput").ap()

    # pair-exchange staging: each core computes K/V for its own 1024 tokens
    # and AllGathers with its batch partner (replica groups of 2)
    PAIRS = [[2 * g, 2 * g + 1] for g in range(NCORES // 2)]
    kxb_d = [nc.dram_tensor(f"kxb{h}", [128, TQ], F16).ap() for h in range(NHP)]
    kg_d = [nc.dram_tensor(f"kg{h}", [2, 128, TQ], F16).ap()
            for h in range(NHP)]
    vxb_d = [[nc.dram_tensor(f"vxb{o}h{h}", [512, 512], F16).ap()
              for h in range(2)] for o in range(2)]
    vg_d = [[nc.dram_tensor(f"vg{o}h{h}", [2, 512, 512], F16).ap()
             for h in range(2)] for o in range(2)]

    xT_r = xT_d.rearrange("(c p) t -> c p t", p=128)
    wq_r = wq_d.rearrange("(c p) o -> p c o", p=128)
    wk_r = wk_d.rearrange("(c p) o -> p c o", p=128)
    wv_r = wv_d.rearrange("(c p) o -> c p o", p=128)
    wo_r = wo_d.rearrange("(c p) o -> c p o", p=128)
    y_r = y_d.rearrange("(tb p) o -> tb p o", p=128)

    NKC = T // 128       # 16 key chunks
    NQB = TQ // 512      # 2 query blocks
    NTB = T // 128       # 16 token blocks for V
    NVO = 2              # V out-feature 512-blocks

    with tile.TileContext(nc) as tc:
        from contextlib import ExitStack
        with ExitStack() as ctx:
            cst = ctx.enter_context(tc.tile_pool(name="cst", bufs=1))
            big = ctx.enter_context(tc.tile_pool(name="big", bufs=1))
            wqk = ctx.enter_context(tc.tile_pool(name="wqk", bufs=2))
            qkp = ctx.enter_context(tc.tile_pool(name="qkp", bufs=2))
            pTp = ctx.enter_context(tc.tile_pool(name="pTp", bufs=14))
            accp = ctx.enter_context(tc.tile_pool(name="accp", bufs=2))
            misc = ctx.enter_context(tc.tile_pool(name="misc", bufs=2))
            outp = ctx.enter_context(tc.tile_pool(name="outp", bufs=3))
            ps_st = ctx.enter_context(
                tc.tile_pool(name="ps_st", bufs=2, space="PSUM"))
            ps_pv = ctx.enter_context(
                tc.tile_pool(name="ps_pv", bufs=2, space="PSUM"))
            ps_sm = ctx.enter_context(
                tc.tile_pool(name="ps_sm", bufs=2, space="PSUM"))

            # ---- persistent tiles ----
            xT = big.tile([128, NIC * TQ], F16, tag="xT")         # 16KB/par
            wv = big.tile([128, NIC * EMBED], F16, tag="wv")      # 16KB
            wo = big.tile([128, NIC * EMBED], F16, tag="wo")      # 16KB
            vv = big.tile([128, NTB * EMBED], F16, tag="vv")      # 32KB
            aT = big.tile([128, NHP * TQ], F16, tag="aT")         # 16KB
            qb_sb = cst.tile([128, NIC], F32, tag="qb")
            kb_sb = cst.tile([128, NIC], F32, tag="kb")
            vbb = cst.tile([128, EMBED], F16, tag="vbb")
            obb = cst.tile([128, EMBED], F32, tag="obb")
            ones = cst.tile([128, 1], F16, tag="ones")
            sel = cst.tile([1, 256], F16, tag="sel")

            for c in range(NIC):
                nc.sync.dma_start(xT[:, c * TQ:(c + 1) * TQ], xT_r[c])
            nc.sync.dma_start(qb_sb[:], qb_d[:])
            nc.sync.dma_start(kb_sb[:], kb_d[:])
            nc.sync.dma_start(ones[:], ones_d[:])
            nc.sync.dma_start(sel[:], sel_d[:])
            nc.sync.dma_start(vbb[:], vbb_d[:])
            nc.sync.dma_start(obb[:], obb_d[:])
            # warm the exp activation table while DMAs stream
            warm = misc.tile([128, 1], F16, tag="warm")
            nc.scalar.activation(warm[:], ones[:], AF.Exp)

            def v_proj_tb(ob, tb):
                # V[tok, feat] for OWN token block tb (<8), feat block ob*512
                ps = ps_sm.tile([128, 512], F32, tag="small")
                for c in range(NIC):
                    nc.tensor.matmul(
                        ps[:],
                        lhsT=xT[:, c * TQ + tb * 128: c * TQ + tb * 128 + 128],
                        rhs=wv[:, c * EMBED + ob * 512: c * EMBED + ob * 512 + 512],
                        start=(c == 0), stop=(c == NIC - 1))
                vtmp = outp.tile([128, 512], F16, tag="vtmp")
                nc.vector.tensor_add(vtmp[:], ps[:],
                                     vbb[:, ob * 512:(ob + 1) * 512])
                nc.gpsimd.dma_start(
                    vxb_d[ob][tb // 4][(tb % 4) * 128:(tb % 4) * 128 + 128, :],
                    vtmp[:])

            def v_gather(ob, half):
                nc.gpsimd.collective_compute(
                    "AllGather", mybir.AluOpType.bypass,
                    ins=[vxb_d[ob][half][:]], outs=[vg_d[ob][half][:]],
                    replica_groups=PAIRS)
                for g in range(2):
                    for j in range(4):
                        gt = g * 8 + half * 4 + j
                        nc.gpsimd.dma_start(
                            vv[:, gt * EMBED + ob * 512:
                               gt * EMBED + ob * 512 + 512],
                            vg_d[ob][half][g, j * 128:(j + 1) * 128, :])

            # ---- per-head-pair K/Q projection, emitted one pair AHEAD,
            # interleaved into the previous pair's attention loop so the
            # scalar engine never drains between pairs ----
            kq = {}

            def alloc_kq(hp):
                wq_sb = wqk.tile([128, NIC * 128], F16, tag="wq")
                wk_sb = wqk.tile([128, NIC * 128], F16, tag="wk")
                nc.sync.dma_start(
                    wq_sb[:].rearrange("p (c o) -> p c o", c=NIC),
                    wq_r[:, :, hp * 128:(hp + 1) * 128])
                nc.sync.dma_start(
                    wk_sb[:].rearrange("p (c o) -> p c o", c=NIC),
                    wk_r[:, :, hp * 128:(hp + 1) * 128])
                kT = qkp.tile([128, T], F16, tag="kT")
                qT = qkp.tile([128, TQ], F16, tag="qT")
                kTh = qkp.tile([128, TQ], F16, tag="kTh")
                kq[hp] = (wq_sb, wk_sb, kT, qT, kTh)

            def k_proj_tb(hp, tb):
                # K^T for OWN tokens only (tb < 2), then pair-exchange
                wq_sb, wk_sb, kT, qT, kTh = kq[hp]
                ps = ps_sm.tile([128, 512], F32, tag="small")
                for c in range(NIC):
                    nc.tensor.matmul(
                        ps[:], lhsT=wk_sb[:, c * 128:(c + 1) * 128],
                        rhs=xT[:, c * TQ + tb * 512: c * TQ + tb * 512 + 512],
                        start=(c == 0), stop=(c == NIC - 1))
                nc.vector.tensor_scalar_add(
                    kTh[:, tb * 512:(tb + 1) * 512], ps[:], kb_sb[:, hp:hp + 1])

            def k_gather(hp):
                _, _, kT, _, kTh = kq[hp]
                nc.gpsimd.dma_start(kxb_d[hp][:], kTh[:])
                nc.gpsimd.collective_compute(
                    "AllGather", mybir.AluOpType.bypass,
                    ins=[kxb_d[hp][:]], outs=[kg_d[hp][:]],
                    replica_groups=PAIRS)
                nc.gpsimd.dma_start(kT[:, 0:TQ], kg_d[hp][0])
                nc.gpsimd.dma_start(kT[:, TQ:T], kg_d[hp][1])

            def q_proj_tb(hp, tb):
                wq_sb, wk_sb, kT, qT, kTh = kq[hp]
                ps = ps_sm.tile([128, 512], F32, tag="small")
                for c in range(NIC):
                    nc.tensor.matmul(
                        ps[:], lhsT=wq_sb[:, c * 128:(c + 1) * 128],
                        rhs=xT[:, c * TQ + tb * 512: c * TQ + tb * 512 + 512],
                        start=(c == 0), stop=(c == NIC - 1))
                nc.vector.tensor_scalar_add(
                    qT[:, tb * 512:(tb + 1) * 512], ps[:], qb_sb[:, hp:hp + 1])

            def o_proj_unit(tb, ob):
                ps = ps_sm.tile([128, 512], F32, tag="small")
                for f in range(NHP):
                    nc.tensor.matmul(
                        ps[:],
                        lhsT=aT[:, f * TQ + tb * 128: f * TQ + tb * 128 + 128],
                        rhs=wo[:, f * EMBED + ob * 512:
                               f * EMBED + ob * 512 + 512],
                        start=(f == 0), stop=(f == NHP - 1))
                out_sb = outp.tile([128, 512], F32, tag="out")
                nc.vector.tensor_add(out_sb[:], ps[:],
                                     obb[:, ob * 512:(ob + 1) * 512])
                nc.gpsimd.dma_start(y_r[tb][:, ob * 512:(ob + 1) * 512],
                                     out_sb[:])

            alloc_kq(0)
            k_proj_tb(0, 0)
            k_proj_tb(0, 1)
            k_gather(0)
            for c in range(NIC):
                nc.sync.dma_start(wv[:, c * EMBED:(c + 1) * EMBED], wv_r[c])
            q_proj_tb(0, 0)
            q_proj_tb(0, 1)
            for tb in range(4):
                v_proj_tb(0, tb)
            v_gather(0, 0)
            for tb in range(4, 8):
                v_proj_tb(0, tb)
            v_gather(0, 1)
            for c in range(NIC):
                nc.sync.dma_start(wo[:, c * EMBED:(c + 1) * EMBED], wo_r[c])

            pending_tail = [None]

            def flush_tail():
                if pending_tail[0] is None:
                    return
                hp_, qb_, pv_, acc_ = pending_tail[0]
                pending_tail[0] = None
                # softmax denominators: ones^T @ acc -> [1, 512] per head
                sums = ps_sm.tile([128, 512], F32, tag="small")
                nc.tensor.matmul(sums[0:1, :], lhsT=ones[:],
                                 rhs=acc_[:, 0:512], start=True, stop=True)
                nc.tensor.matmul(sums[32:33, :], lhsT=ones[:],
                                 rhs=acc_[:, 512:1024], start=True,
                                 stop=True, tile_position=(0, 32))
                sums_sb = misc.tile([1, 1024], F16, tag="sums_sb")
                with nc.allow_low_precision(
                        reason="softmax denominators, fp16 ample"):
                    nc.vector.tensor_copy(
                        sums_sb[:, 0:512], sums[0:1, 0:512])
                    nc.vector.tensor_copy(
                        sums_sb[:, 512:1024], sums[32:33, 0:512])
                bc = ps_sm.tile([128, 512], F32, tag="small")
                nc.tensor.matmul(bc[:], lhsT=sel[:, 0:128],
                                 rhs=sums_sb[:, 0:512], start=True,
                                 stop=False)
                nc.tensor.matmul(bc[:], lhsT=sel[:, 128:256],
                                 rhs=sums_sb[:, 512:1024], start=False,
                                 stop=True)
                bc_sb = misc.tile([128, 512], F32, tag="bc_sb")
                nc.vector.reciprocal_approx_fast(bc_sb[:], bc[:])
                nc.vector.tensor_mul(
                    aT[:, hp_ * TQ + qb_ * 512: hp_ * TQ + qb_ * 512 + 512],
                    pv_[:], bc_sb[:])

            for hp in range(NHP):
                _, _, kT, qT, _ = kq[hp]

                for qb in range(NQB):
                    if hp + 1 < NHP and qb == 0:
                        alloc_kq(hp + 1)
                    # interleave units: next-pair K/Q proj, the second V
                    # feature-block under hp1, and the first half of the
                    # out-projection under hp7 qb1
                    units = {}
                    if hp + 1 < NHP:
                        if qb == 0:
                            units = {1: (k_proj_tb, hp + 1, 0),
                                     5: (k_proj_tb, hp + 1, 1),
                                     9: (k_gather, hp + 1, None)}
                        else:
                            units = {2: (q_proj_tb, hp + 1, 0),
                                     10: (q_proj_tb, hp + 1, 1)}
                    elif qb == 1:
                        units = {k: (o_proj_unit, u // 2, u % 2)
                                 for u, k in enumerate(
                                     [3, 4, 5, 6, 7, 9, 11, 13])}
                    if hp == 1:
                        if qb == 0:
                            units[3] = (v_proj_tb, 1, 0)
                            units[7] = (v_proj_tb, 1, 1)
                            units[11] = (v_proj_tb, 1, 2)
                            units[13] = (v_proj_tb, 1, 3)
                            units[15] = (v_gather, 1, 0)
                        else:
                            units[4] = (v_proj_tb, 1, 4)
                            units[6] = (v_proj_tb, 1, 5)
                            units[8] = (v_proj_tb, 1, 6)
                            units[12] = (v_proj_tb, 1, 7)
                            units[14] = (v_gather, 1, 1)
                    pv = ps_pv.tile([128, 512], F32, tag="pv")
                    acc = accp.tile([128, 1024], F16, tag="acc")
                    prev_pT = None
                    for kc in range(NKC):
                        if kc == 2:
                            flush_tail()
                        if kc in units:
                            fn, a0, a1 = units[kc]
                            if a1 is None:
                                fn(a0)
                            else:
                                fn(a0, a1)
                        st = ps_st.tile([128, 1024], F32, tag="st")
                        nc.tensor.matmul(
                            st[:, 0:512],
                            lhsT=kT[0:64, kc * 128:(kc + 1) * 128],
                            rhs=qT[0:64, qb * 512:(qb + 1) * 512],
                            start=True, stop=True)
                        nc.tensor.matmul(
                            st[:, 512:1024],
                            lhsT=kT[64:128, kc * 128:(kc + 1) * 128],
                            rhs=qT[64:128, qb * 512:(qb + 1) * 512],
                            start=True, stop=True, tile_position=(64, 0))
                        pT = pTp.tile([128, 1024], F16, tag="pT")
                        nc.scalar.activation(pT[:], st[:], AF.Exp)
                        with nc.allow_low_precision(
                                reason="fp16 softmax partial-sum accumulate"):
                            if kc == 1:
                                nc.vector.tensor_add(
                                    acc[:], prev_pT[:], pT[:])
                            elif kc > 1:
                                nc.vector.tensor_add(acc[:], acc[:], pT[:])
                        prev_pT = pT
                        nc.tensor.matmul(
                            pv[0:64, :],
                            lhsT=vv[:, kc * EMBED + hp * 128:
                                    kc * EMBED + hp * 128 + 64],
                            rhs=pT[:, 0:512],
                            start=(kc == 0), stop=(kc == NKC - 1))
                        nc.tensor.matmul(
                            pv[64:128, :],
                            lhsT=vv[:, kc * EMBED + hp * 128 + 64:
                                    kc * EMBED + hp * 128 + 128],
                            rhs=pT[:, 512:1024],
                            start=(kc == 0), stop=(kc == NKC - 1),
                            tile_position=(0, 64))

                    # defer this iteration's softmax tail so the next
                    # iteration's first scores/exp keep ScalarE fed
                    pending_tail[0] = (hp, qb, pv, acc)

            flush_tail()

            # ---- remaining out projection (qb1 token blocks) ----
            for tb in range(TQ // 256, TQ // 128):
                for ob in range(2):
                    o_proj_unit(tb, ob)

    nc.compile()
    return nc


def _get_program():
    global _PROGRAM
    if _PROGRAM is None:
        _PROGRAM = _build_program()
    return _PROGRAM


def _make_in_maps(x, q_w, q_b, k_w, k_b, v_w, v_b, o_w, o_b):
    f16 = np.float16
    # softmax scale folded into the Q projection
    wqT = np.ascontiguousarray((q_w.astype(np.float32).T / 8.0)).astype(f16)
    wkT = np.ascontiguousarray(k_w.astype(np.float32).T).astype(f16)
    wvT = np.ascontiguousarray(v_w.astype(np.float32).T).astype(f16)
    woT = np.ascontiguousarray(o_w.astype(np.float32).T).astype(f16)
    qb = np.ascontiguousarray(
        (q_b.astype(np.float32) / 8.0).reshape(NIC, 128).T)
    kb = np.ascontiguousarray(k_b.astype(np.float32).reshape(NIC, 128).T)
    vbb = np.broadcast_to(v_b.astype(np.float32), (128, EMBED)).astype(f16)
    vbb = np.ascontiguousarray(vbb)
    obb = np.ascontiguousarray(
        np.broadcast_to(o_b.astype(np.float32), (128, EMBED)))
    ones = np.ones((128, 1), f16)
    sel = np.zeros((1, 256), f16)
    sel[0, 0:64] = 1.0
    sel[0, 192:256] = 1.0
    in_maps = []
    for c in range(NCORES):
        b, qh = c // 2, c % 2
        # own-token slab only; the partner's K/V arrive via the pair
        # AllGather on device (k ordering is irrelevant to softmax)
        xT = np.ascontiguousarray(
            x[b, qh * TQ:(qh + 1) * TQ].astype(np.float32).T).astype(f16)
        in_maps.append({
            "xT": xT, "wqT": wqT, "wkT": wkT, "wvT": wvT, "woT": woT,
            "qb": qb, "kb": kb, "vbb": vbb, "obb": obb,
            "ones": ones, "sel": sel,
        })
    return in_maps


def kernel(x, mask, q_w, q_b, k_w, k_b, v_w, v_b, o_w, o_b):
    from concourse.bass_utils import run_bass_kernel_spmd

    nc = _get_program()
    x = np.asarray(x)
    in_maps = _make_in_maps(np.asarray(x), np.asarray(q_w), np.asarray(q_b),
                            np.asarray(k_w), np.asarray(k_b),
                            np.asarray(v_w), np.asarray(v_b),
                            np.asarray(o_w), np.asarray(o_b))
    res = run_bass_kernel_spmd(nc, in_maps, list(range(NCORES)))
    out = np.empty((B, T, EMBED), np.float32)
    for c in range(NCORES):
        b, qh = c // 2, c % 2
        out[b, qh * TQ:(qh + 1) * TQ, :] = res.results[c]["y"]
    return out
